# revision 9
# baseline (speedup 1.0000x reference)
"""WaveNet-style gated residual block (AdvancedSkipResidualBlock) on 8 TRN2 NeuronCores.

Strategy: data-parallel over batch B=8 -> one batch element per NeuronCore.
Per core the block is 7 GEMMs of [512,512] x [512,T]; the two dilated causal
convs (K=2, d=2) are restructured with Winograd F(2,2) so their GEMM column
count drops 25% (total matmul cycles -14%):

Time axis is permuted host-side into 4 phases (t mod 4) so each chunk holds
4 contiguous phase blocks of J pair-columns. For the even chain
(y[4j], y[4j+2]) and odd chain (y[4j+1], y[4j+3]):
  m1 = (W0+W1) @ ph01[j]          (shared by both outputs of the pair)
  m2 = W0 @ (ph23[j-1] - ph01[j])
  m3 = W1 @ (ph23[j]   - ph01[j])
  y(ph01) = m1 + m2   (PE accumulates m2 in-place onto m1's PSUM bank)
  y(ph23) = m1 + m3   (m1 copied to SBUF between the groups; DVE adds)
All 1x1 GEMMs (cond/res/skip) are column-order invariant so they run on the
permuted layout unchanged; outputs are un-permuted host-side (free for HW).

Matmuls in bf16 (fp32 PSUM), N>=256 per matmul so LDWEIGHTS stays hidden.
I/O rides a host-side per-chunk-blocked layout ([G,4,J] blocks per
partition row) so every chunk DMA is 128 segments of multi-KB. All startup
DMAs ride the sync queue in exact need-order (single queue = strict
transfer ordering; cross-queue deps do not gate transfers), and dummy
matmuls bridge the preamble to the first real bytes so the HAM clock gate
opens once and stays open. Measured ~187.5us NEFF exec (bf16 matmul
issue-floor for this structure ~166.5us + startup DMA + drain).
"""

import os
import sys
from contextlib import ExitStack

import numpy as np

try:
    import concourse.bass as bass  # noqa: F401
except ImportError:  # pragma: no cover
    sys.path.insert(0, "/opt/trn_rl_repo")
    import concourse.bass as bass  # noqa: F401

import ml_dtypes  # noqa: E402
import concourse.tile as tile  # noqa: E402
from concourse import bacc, mybir  # noqa: E402
from concourse.bass_utils import run_bass_kernel_spmd  # noqa: E402

B, C, T = 8, 512, 4096
P, G = 128, 4          # SBUF partitions, channel groups (C = G*P)
Q = T // 4             # pair-columns per phase
# time-cols per chunk (mult of 4). N-per-matmul is 2*J = w/2; at 512 cols
# and above the per-column matmul issue cost is flat (~0.42ns/col), below
# that LDWEIGHTS dominates. Two 512 head chunks keep the startup DMA
# critical path short while weights stream.
CWS = [512, 1024, 1024, 1024, 512]
assert sum(CWS) == T
JS = [w // 4 for w in CWS]                # pair-cols per chunk
JOS = [0]
for _j in JS:
    JOS.append(JOS[-1] + _j)
NCH = len(CWS)

BF16 = mybir.dt.bfloat16
F32 = mybir.dt.float32
AF = mybir.ActivationFunctionType
ALU = mybir.AluOpType

_CACHE: dict = {}


def _build():
    nc = bacc.Bacc("TRN2", target_bir_lowering=False, debug=False, num_devices=B)

    def din(name, shape, dt):
        return nc.dram_tensor(name, shape, dt, kind="ExternalInput").ap()

    def dout(name, shape, dt):
        return nc.dram_tensor(name, shape, dt, kind="ExternalOutput").ap()

    # host pre-arranged per-chunk-blocked: for chunk c the cols
    # [16*JOS[c] : 16*JOS[c]+16*J] hold the [G, 4, J] block (g, t%4, pair)
    # of partition p. Chunk DMAs are then 128 segments of 16*J*2B.
    x_d = din("x", [P, 4 * G * Q], BF16)
    c_d = din("cond", [P, 4 * G * Q], BF16)
    wc_d = din("wc", [P, G, G, P], BF16)
    wfs_d = din("wfs", [P, G, G, P], BF16)  # Wf0+Wf1
    wf0_d = din("wf0", [P, G, G, P], BF16)
    wf1_d = din("wf1", [P, G, G, P], BF16)
    wgs_d = din("wgs", [P, G, G, P], BF16)
    wg0_d = din("wg0", [P, G, G, P], BF16)
    wg1_d = din("wg1", [P, G, G, P], BF16)
    wr_d = din("wr", [P, G, G, P], BF16)
    ws_d = din("ws", [P, G, G, P], BF16)
    bias_d = din("bias", [P, 5 * G], F32)
    r_d = dout("res", [P, 4 * G * Q], BF16)
    s_d = dout("skip", [P, 4 * G * Q], BF16)

    x_r, c_r, r_r, s_r = x_d, c_d, r_d, s_d

    with tile.TileContext(nc) as tc, ExitStack() as ctx:
        const = ctx.enter_context(tc.tile_pool(name="const", bufs=1))
        cin = ctx.enter_context(tc.tile_pool(name="cin", bufs=3))
        xin = ctx.enter_context(tc.tile_pool(name="xin", bufs=3))
        xcp = ctx.enter_context(tc.tile_pool(name="xcp", bufs=3))
        dpl = ctx.enter_context(tc.tile_pool(name="dpl", bufs=4))
        m1p = ctx.enter_context(tc.tile_pool(name="m1p", bufs=4))
        fgp = ctx.enter_context(tc.tile_pool(name="fgp", bufs=8))
        hp = ctx.enter_context(tc.tile_pool(name="hp", bufs=3))
        rop = ctx.enter_context(tc.tile_pool(name="rop", bufs=3))
        sop = ctx.enter_context(tc.tile_pool(name="sop", bufs=3))
        psum = ctx.enter_context(tc.tile_pool(name="psum", bufs=8, space="PSUM"))

        # ---- PE warm-up: dummy matmuls bridge the runtime-preamble end to
        # the arrival of the first real bytes (~wc+ct0), so the HAM clock
        # gate opens before the real stream starts and stays open.
        dummy = const.tile([P, 512], BF16)
        nc.vector.memset(dummy[:], 0.0)
        psD = psum.tile([P, 512], F32, space="PSUM", tag="ps")
        for _ in range(5):
            nc.tensor.matmul(psD, dummy[:, 0:128], dummy[:], start=True, stop=True)

        # ---- Startup DMA choreography: EVERYTHING on the sync queue in
        # exact need-order. One queue = strict transfer ordering, so no
        # stream can steal HBM bandwidth from bytes that gate earlier PE
        # work (cross-queue instruction deps do NOT gate transfers).
        def ct_in(c):
            J, jo = JS[c], JOS[c]
            t = cin.tile([P, G, 4, J], BF16, tag="c")
            nc.sync.dma_start(t[:], c_r[:, 16 * jo:16 * (jo + J)])
            return t

        def xt_in(c):
            J, jo = JS[c], JOS[c]
            t = xin.tile([P, G, 4, J], BF16, tag="x")
            nc.sync.dma_start(t[:], x_r[:, 16 * jo:16 * (jo + J)])
            return t

        def chunk_in(c):
            return ct_in(c), xt_in(c)

        # Need-order for the PE stream: st1(c0) gates on wc+ct0, st1(c1) on
        # ct1, st2(c0) m1 on xt0+bias+wfs/wgs, m3 on wf1/wg1, m2 on wf0/wg0.
        # xt1 is only read by the DVE STT of st1(c1), so it loads after the
        # conv weights.
        wc_h = []
        for h in range(2):
            t = const.tile([P, 2, G, P], BF16, tag=f"wc{h}")
            wc_h.append(t)
        nc.sync.dma_start(wc_h[0][:], wc_d[:, 0:2])
        ct0 = ct_in(0)
        nc.sync.dma_start(wc_h[1][:], wc_d[:, 2:4])
        ct1 = ct_in(1)
        b_all = const.tile([P, 5 * G], F32)
        nc.sync.dma_start(b_all[:], bias_d)
        xt0 = xt_in(0)
        pre_c, pre_x = {0: ct0, 1: ct1}, {0: xt0}

        def wload(name, dram):
            t = const.tile([P, G, G, P], BF16, tag=name)
            nc.sync.dma_start(t[:], dram)
            return t

        wfs_sb = wload("wfs", wfs_d)
        wgs_sb = wload("wgs", wgs_d)
        wf1_sb = wload("wf1", wf1_d)
        wg1_sb = wload("wg1", wg1_d)
        pre_x[1] = xt_in(1)
        wf0_sb = wload("wf0", wf0_d)
        wg0_sb = wload("wg0", wg0_d)
        pre_c[2], pre_x[2] = chunk_in(2)
        wr_sb = wload("wr", wr_d)
        ws_sb = wload("ws", ws_d)
        wsum = (wfs_sb, wgs_sb)
        wtap0 = (wf0_sb, wg0_sb)
        wtap1 = (wf1_sb, wg1_sb)
        _bidx = {"bc": 0, "bf": 1, "bg": 2, "br": 3, "bs": 4}

        def bias_ap(name, m):
            return b_all[:, _bidx[name] * G + m:_bidx[name] * G + m + 1]

        xc_t: dict = {}
        h_t: dict = {}
        for it in range(NCH + 2):
            c0, c1, c2 = it, it - 1, it - 2

            # ---- stage 1: condition injection (chunk c0) ----
            # xc[:, m, ph, 1+j] = x + Wc@cond + bc at t = 4*(jo+j)+ph;
            # col 0 of each phase is the halo (previous chunk's last pair).
            if c0 < NCH:
                J, jo = JS[c0], JOS[c0]
                if c0 in pre_c:
                    ct, xt = pre_c.pop(c0), pre_x.pop(c0)
                else:
                    ct, xt = chunk_in(c0)
                xc = xcp.tile([P, G, 4, J + 1], BF16, tag="xc")
                if c0 == 0:
                    nc.vector.memset(xc[:, :, :, 0:1], 0.0)
                else:
                    Jp = JS[c0 - 1]
                    nc.vector.tensor_copy(xc[:, :, :, 0:1],
                                          xc_t[c0 - 1][:, :, :, Jp:Jp + 1])
                for m in range(G):
                    for h2 in range(2):
                        ps = psum.tile([P, 2, J], F32, space="PSUM", tag="ps")
                        for k in range(G):
                            nc.tensor.matmul(ps, wc_h[k // 2][:, k % 2, m, :],
                                             ct[:, k, 2 * h2:2 * h2 + 2, :],
                                             start=(k == 0), stop=(k == G - 1))
                        nc.vector.scalar_tensor_tensor(
                            xc[:, m, 2 * h2:2 * h2 + 2, 1:J + 1], ps,
                            bias_ap("bc", m), xt[:, m, 2 * h2:2 * h2 + 2, :],
                            ALU.add, ALU.add)
                xc_t[c0] = xc

            # ---- stage 2: Winograd dilated conv + gated activation (c1) ----
            if 0 <= c1 < NCH:
                J = JS[c1]
                xc = xc_t[c1]
                d2 = dpl.tile([P, G, 2, J], BF16, tag="d2")
                nc.vector.tensor_sub(d2[:], xc[:, :, 2:4, 0:J],
                                     xc[:, :, 0:2, 1:J + 1])
                d3 = dpl.tile([P, G, 2, J], BF16, tag="d3")
                nc.vector.tensor_sub(d3[:], xc[:, :, 2:4, 1:J + 1],
                                     xc[:, :, 0:2, 1:J + 1])
                h = hp.tile([P, G, 4, J], BF16, tag="h")
                for m in range(G):
                    pA, pB, m1sb = [], [], []
                    # m1 = Wsum @ ph01 for both convs
                    for cv in range(2):
                        ps = psum.tile([P, 2, J], F32, space="PSUM", tag="ps")
                        for k in range(G):
                            nc.tensor.matmul(ps, wsum[cv][:, k, m, :],
                                             xc[:, k, 0:2, 1:J + 1],
                                             start=(k == 0), stop=(k == G - 1))
                        pA.append(ps)
                    # snapshot m1 (needed again for the ph23 outputs)
                    for cv in range(2):
                        t = m1p.tile([P, 2, J], BF16, tag="m1")
                        nc.vector.tensor_copy(t[:], pA[cv])
                        m1sb.append(t)
                    # m3 = Wtap1 @ d3 (fresh banks) -- gives the copies slack
                    for cv in range(2):
                        ps = psum.tile([P, 2, J], F32, space="PSUM", tag="ps")
                        for k in range(G):
                            nc.tensor.matmul(ps, wtap1[cv][:, k, m, :], d3[:, k],
                                             start=(k == 0), stop=(k == G - 1))
                        pB.append(ps)
                    # m2 = Wtap0 @ d2 accumulated in-place onto m1 -> y(ph01)
                    for cv in range(2):
                        for k in range(G):
                            nc.tensor.matmul(pA[cv], wtap0[cv][:, k, m, :],
                                             d2[:, k], start=False,
                                             stop=(k == G - 1),
                                             skip_group_check=True)
                    fg01, fg23 = [], []
                    for cv, afn, bn in ((0, AF.Tanh, "bf"),
                                        (1, AF.Sigmoid, "bg")):
                        t01 = fgp.tile([P, 2, J], BF16, tag="fg")
                        nc.scalar.activation(t01, pA[cv], afn,
                                             bias=bias_ap(bn, m))
                        p23 = fgp.tile([P, 2, J], BF16, tag="fg")
                        nc.vector.tensor_add(p23, pB[cv], m1sb[cv])
                        t23 = fgp.tile([P, 2, J], BF16, tag="fg")
                        nc.scalar.activation(t23, p23, afn,
                                             bias=bias_ap(bn, m))
                        fg01.append(t01)
                        fg23.append(t23)
                    nc.vector.tensor_mul(h[:, m, 0:2, :], fg01[0], fg01[1])
                    nc.vector.tensor_mul(h[:, m, 2:4, :], fg23[0], fg23[1])
                h_t[c1] = h

            # ---- stage 3: residual + skip projections (chunk c2) ----
            if 0 <= c2 < NCH:
                J, jo = JS[c2], JOS[c2]
                h = h_t.pop(c2)
                xc = xc_t.pop(c2)
                # per-half output DMAs on the last chunk start the final
                # stores before the second half's compute finishes
                fine = (c2 == NCH - 1)
                for m in range(G):
                    rt = rop.tile([P, 4, J], BF16, tag="r")
                    st = sop.tile([P, 4, J], BF16, tag="s")
                    off = 16 * jo + m * 4 * J
                    for h2 in range(2):
                        sl = slice(2 * h2, 2 * h2 + 2)
                        osl = slice(off + 2 * h2 * J, off + (2 * h2 + 2) * J)
                        pr = psum.tile([P, 2, J], F32, space="PSUM", tag="ps")
                        for k in range(G):
                            nc.tensor.matmul(pr, wr_sb[:, k, m, :],
                                             h[:, k, sl, :],
                                             start=(k == 0), stop=(k == G - 1))
                        nc.vector.scalar_tensor_tensor(
                            rt[:, sl, :], pr, bias_ap("br", m),
                            xc[:, m, sl, 1:J + 1], ALU.add, ALU.add)
                        pk = psum.tile([P, 2, J], F32, space="PSUM", tag="ps")
                        for k in range(G):
                            nc.tensor.matmul(pk, ws_sb[:, k, m, :],
                                             h[:, k, sl, :],
                                             start=(k == 0), stop=(k == G - 1))
                        nc.scalar.activation(st[:, sl, :], pk, AF.Identity,
                                             bias=bias_ap("bs", m))
                        if fine:
                            nc.scalar.dma_start(r_r[:, osl], rt[:, sl, :])
                            nc.gpsimd.dma_start(s_r[:, osl], st[:, sl, :])
                    if not fine:
                        nc.scalar.dma_start(r_r[:, off:off + 4 * J], rt[:])
                        nc.gpsimd.dma_start(s_r[:, off:off + 4 * J], st[:])

    nc.compile()
    return nc


def _get_nc():
    if "nc" not in _CACHE:
        _CACHE["nc"] = _build()
    return _CACHE["nc"]


def _wT1(w):
    # [Cout, Cin] -> lhsT layout [P(cin%P), G(cin//P), G(cout//P), P(cout%P)]
    return np.ascontiguousarray(
        np.asarray(w, dtype=np.float32).T.reshape(G, P, G, P)
        .transpose(1, 0, 2, 3).astype(ml_dtypes.bfloat16))


def _bias(b):
    return np.ascontiguousarray(np.asarray(b).reshape(G, P).T.astype(np.float32))


def _phase_perm(a):
    # [B, C, T] -> [B, P, 16Q] per-chunk-blocked: chunk c holds the
    # [G, 4, J_c] block (g, t%4, pair) flattened, chunks concatenated.
    ap = (np.asarray(a).reshape(B, G, P, Q, 4).transpose(0, 2, 1, 4, 3)
          .astype(ml_dtypes.bfloat16))          # [B, P, G, 4, Q]
    blocks = [
        np.ascontiguousarray(ap[:, :, :, :, JOS[c]:JOS[c] + JS[c]])
        .reshape(B, P, 16 * JS[c])
        for c in range(NCH)
    ]
    return np.ascontiguousarray(np.concatenate(blocks, axis=2))


def kernel(x, condition, wf, bf, wg, bg, wr, br, ws, bs, wc, bc):
    nc = _get_nc()
    x_p = _phase_perm(x)
    c_p = _phase_perm(condition)
    wf = np.asarray(wf, dtype=np.float32)
    wg = np.asarray(wg, dtype=np.float32)
    shared = {
        "wc": _wT1(np.asarray(wc)[:, :, 0]),
        "wfs": _wT1(wf[:, :, 0] + wf[:, :, 1]),
        "wf0": _wT1(wf[:, :, 0]),
        "wf1": _wT1(wf[:, :, 1]),
        "wgs": _wT1(wg[:, :, 0] + wg[:, :, 1]),
        "wg0": _wT1(wg[:, :, 0]),
        "wg1": _wT1(wg[:, :, 1]),
        "wr": _wT1(np.asarray(wr)[:, :, 0]),
        "ws": _wT1(np.asarray(ws)[:, :, 0]),
        "bias": np.ascontiguousarray(np.concatenate(
            [_bias(b) for b in (bc, bf, bg, br, bs)], axis=1)),
    }
    in_maps = [
        {"x": np.ascontiguousarray(x_p[i]), "cond": np.ascontiguousarray(c_p[i]),
         **shared}
        for i in range(B)
    ]
    res = run_bass_kernel_spmd(
        nc, in_maps, list(range(B)),
        trace=bool(os.environ.get("CC_KERNEL_TRACE")))
    _CACHE["last_results"] = res

    def unperm(name):
        # per-chunk-blocked [P, 16Q] -> [C, T]
        out = np.empty((B, P, G, 4, Q), dtype=np.float32)
        for i in range(B):
            flat = np.asarray(res.results[i][name]).astype(np.float32)
            for c in range(NCH):
                jo, J = JOS[c], JS[c]
                out[i, :, :, :, jo:jo + J] = (
                    flat[:, 16 * jo:16 * (jo + J)].reshape(P, G, 4, J))
        return np.ascontiguousarray(
            out.transpose(0, 2, 1, 4, 3).reshape(B, C, T))

    return unperm("res"), unperm("skip")



# revision 13
# speedup vs baseline: 1.0178x; 1.0178x over previous
"""WaveNet-style gated residual block (AdvancedSkipResidualBlock) on 8 TRN2 NeuronCores.

Strategy: data-parallel over batch B=8 -> one batch element per NeuronCore.
Per core the block is 7 GEMMs of [512,512] x [512,T]; the two dilated causal
convs (K=2, d=2) are restructured with Winograd F(2,2) so their GEMM column
count drops 25% (total matmul cycles -14%):

Time axis is permuted host-side into 4 phases (t mod 4) so each chunk holds
4 contiguous phase blocks of J pair-columns. For the even chain
(y[4j], y[4j+2]) and odd chain (y[4j+1], y[4j+3]):
  m1 = (W0+W1) @ ph01[j]          (shared by both outputs of the pair)
  m2 = W0 @ (ph23[j-1] - ph01[j])
  m3 = W1 @ (ph23[j]   - ph01[j])
  y(ph01) = m1 + m2   (PE accumulates m2 in-place onto m1's PSUM bank)
  y(ph23) = m1 + m3   (m1 copied to SBUF between the groups; DVE adds)
All 1x1 GEMMs (cond/res/skip) are column-order invariant so they run on the
permuted layout unchanged; outputs are un-permuted host-side (free for HW).

Matmuls in bf16 (fp32 PSUM), N>=256 per matmul so LDWEIGHTS stays hidden.
I/O rides a host-side per-chunk-blocked layout ([G,4,J] blocks per
partition row) so every chunk DMA is 128 segments of multi-KB. All startup
DMAs ride the sync queue in exact need-order (single queue = strict
transfer ordering; cross-queue deps do not gate transfers), and dummy
matmuls bridge the preamble to the first real bytes so the HAM clock gate
opens once and stays open. Measured ~187.5us NEFF exec (bf16 matmul
issue-floor for this structure ~166.5us + startup DMA + drain).
"""

import os
import sys
from contextlib import ExitStack

import numpy as np

try:
    import concourse.bass as bass  # noqa: F401
except ImportError:  # pragma: no cover
    sys.path.insert(0, "/opt/trn_rl_repo")
    import concourse.bass as bass  # noqa: F401

import ml_dtypes  # noqa: E402
import concourse.tile as tile  # noqa: E402
from concourse import bacc, mybir  # noqa: E402
from concourse.bass_utils import run_bass_kernel_spmd  # noqa: E402

B, C, T = 8, 512, 4096
P, G = 128, 4          # SBUF partitions, channel groups (C = G*P)
Q = T // 4             # pair-columns per phase
# time-cols per chunk (mult of 4). N-per-matmul is 2*J = w/2; at 512 cols
# and above the per-column matmul issue cost is flat (~0.42ns/col), below
# that LDWEIGHTS dominates. Two 512 head chunks keep the startup DMA
# critical path short while weights stream.
CWS = [512, 512, 1024, 1024, 1024]
assert sum(CWS) == T
JS = [w // 4 for w in CWS]                # pair-cols per chunk
JOS = [0]
for _j in JS:
    JOS.append(JOS[-1] + _j)
NCH = len(CWS)

BF16 = mybir.dt.bfloat16
F32 = mybir.dt.float32
AF = mybir.ActivationFunctionType
ALU = mybir.AluOpType

_CACHE: dict = {}


def _build():
    nc = bacc.Bacc("TRN2", target_bir_lowering=False, debug=False, num_devices=B)

    def din(name, shape, dt):
        return nc.dram_tensor(name, shape, dt, kind="ExternalInput").ap()

    def dout(name, shape, dt):
        return nc.dram_tensor(name, shape, dt, kind="ExternalOutput").ap()

    # host pre-arranged per-chunk-blocked: for chunk c the cols
    # [16*JOS[c] : 16*JOS[c]+16*J] hold the [G, 4, J] block (g, t%4, pair)
    # of partition p. Chunk DMAs are then 128 segments of 16*J*2B.
    x_d = din("x", [P, 4 * G * Q], BF16)
    c_d = din("cond", [P, 4 * G * Q], BF16)
    wc_d = din("wc", [P, G, G, P], BF16)
    wfs_d = din("wfs", [P, G, G, P], BF16)  # Wf0+Wf1
    wf0_d = din("wf0", [P, G, G, P], BF16)
    wf1_d = din("wf1", [P, G, G, P], BF16)
    wgs_d = din("wgs", [P, G, G, P], BF16)
    wg0_d = din("wg0", [P, G, G, P], BF16)
    wg1_d = din("wg1", [P, G, G, P], BF16)
    wr_d = din("wr", [P, G, G, P], BF16)
    ws_d = din("ws", [P, G, G, P], BF16)
    bias_d = din("bias", [P, 5 * G], F32)
    r_d = dout("res", [P, 4 * G * Q], BF16)
    s_d = dout("skip", [P, 4 * G * Q], BF16)

    x_r, c_r, r_r, s_r = x_d, c_d, r_d, s_d

    with tile.TileContext(nc) as tc, ExitStack() as ctx:
        const = ctx.enter_context(tc.tile_pool(name="const", bufs=1))
        cin = ctx.enter_context(tc.tile_pool(name="cin", bufs=3))
        xin = ctx.enter_context(tc.tile_pool(name="xin", bufs=3))
        xcp = ctx.enter_context(tc.tile_pool(name="xcp", bufs=3))
        dpl = ctx.enter_context(tc.tile_pool(name="dpl", bufs=4))
        m1p = ctx.enter_context(tc.tile_pool(name="m1p", bufs=4))
        fgp = ctx.enter_context(tc.tile_pool(name="fgp", bufs=8))
        hp = ctx.enter_context(tc.tile_pool(name="hp", bufs=3))
        rop = ctx.enter_context(tc.tile_pool(name="rop", bufs=3))
        sop = ctx.enter_context(tc.tile_pool(name="sop", bufs=3))
        psum = ctx.enter_context(tc.tile_pool(name="psum", bufs=8, space="PSUM"))

        # ---- PE warm-up: dummy matmuls bridge the runtime-preamble end to
        # the arrival of the first real bytes (~wc+ct0), so the HAM clock
        # gate opens before the real stream starts and stays open.
        dummy = const.tile([P, 512], BF16)
        nc.vector.memset(dummy[:], 0.0)
        psD = psum.tile([P, 512], F32, space="PSUM", tag="ps")
        for _ in range(16):
            nc.tensor.matmul(psD, dummy[:, 0:128], dummy[:], start=True, stop=True)

        # ---- Startup DMA choreography: EVERYTHING on the sync queue in
        # exact need-order. One queue = strict transfer ordering, so no
        # stream can steal HBM bandwidth from bytes that gate earlier PE
        # work (cross-queue instruction deps do NOT gate transfers).
        def ct_in(c):
            J, jo = JS[c], JOS[c]
            t = cin.tile([P, G, 4, J], BF16, tag="c")
            nc.sync.dma_start(t[:], c_r[:, 16 * jo:16 * (jo + J)])
            return t

        def xt_in(c):
            J, jo = JS[c], JOS[c]
            t = xin.tile([P, G, 4, J], BF16, tag="x")
            nc.sync.dma_start(t[:], x_r[:, 16 * jo:16 * (jo + J)])
            return t

        def chunk_in(c):
            return ct_in(c), xt_in(c)

        # Startup choreography (need-order on the single sync queue):
        # wc in two halves so the first accumulation chain starts on k=0,1.
        wc_h = []
        for h in range(2):
            t = const.tile([P, 2, G, P], BF16, tag=f"wc{h}")
            wc_h.append(t)
        nc.sync.dma_start(wc_h[0][:], wc_d[:, 0:2])
        ct0 = ct_in(0)
        nc.sync.dma_start(wc_h[1][:], wc_d[:, 2:4])
        xt0 = xt_in(0)
        b_all = const.tile([P, 5 * G], F32)
        nc.sync.dma_start(b_all[:], bias_d)
        pre_c, pre_x = {0: ct0}, {0: xt0}

        def wload(name, dram):
            t = const.tile([P, G, G, P], BF16, tag=name)
            nc.sync.dma_start(t[:], dram)
            return t

        wfs_sb = wload("wfs", wfs_d)
        pre_c[1], pre_x[1] = chunk_in(1)
        wgs_sb = wload("wgs", wgs_d)
        wf1_sb = wload("wf1", wf1_d)
        wg1_sb = wload("wg1", wg1_d)
        wf0_sb = wload("wf0", wf0_d)
        wg0_sb = wload("wg0", wg0_d)
        pre_c[2], pre_x[2] = chunk_in(2)
        wr_sb = wload("wr", wr_d)
        ws_sb = wload("ws", ws_d)
        wsum = (wfs_sb, wgs_sb)
        wtap0 = (wf0_sb, wg0_sb)
        wtap1 = (wf1_sb, wg1_sb)
        _bidx = {"bc": 0, "bf": 1, "bg": 2, "br": 3, "bs": 4}

        def bias_ap(name, m):
            return b_all[:, _bidx[name] * G + m:_bidx[name] * G + m + 1]

        xc_t: dict = {}
        h_t: dict = {}
        for it in range(NCH + 2):
            c0, c1, c2 = it, it - 1, it - 2

            # ---- stage 1: condition injection (chunk c0) ----
            # xc[:, m, ph, 1+j] = x + Wc@cond + bc at t = 4*(jo+j)+ph;
            # col 0 of each phase is the halo (previous chunk's last pair).
            if c0 < NCH:
                J, jo = JS[c0], JOS[c0]
                if c0 in pre_c:
                    ct, xt = pre_c.pop(c0), pre_x.pop(c0)
                else:
                    ct, xt = chunk_in(c0)
                xc = xcp.tile([P, G, 4, J + 1], BF16, tag="xc")
                if c0 == 0:
                    nc.vector.memset(xc[:, :, :, 0:1], 0.0)
                else:
                    Jp = JS[c0 - 1]
                    nc.vector.tensor_copy(xc[:, :, :, 0:1],
                                          xc_t[c0 - 1][:, :, :, Jp:Jp + 1])
                for m in range(G):
                    for h2 in range(2):
                        ps = psum.tile([P, 2, J], F32, space="PSUM", tag="ps")
                        for k in range(G):
                            nc.tensor.matmul(ps, wc_h[k // 2][:, k % 2, m, :],
                                             ct[:, k, 2 * h2:2 * h2 + 2, :],
                                             start=(k == 0), stop=(k == G - 1))
                        nc.vector.scalar_tensor_tensor(
                            xc[:, m, 2 * h2:2 * h2 + 2, 1:J + 1], ps,
                            bias_ap("bc", m), xt[:, m, 2 * h2:2 * h2 + 2, :],
                            ALU.add, ALU.add)
                xc_t[c0] = xc

            # ---- stage 2: Winograd dilated conv + gated activation (c1) ----
            if 0 <= c1 < NCH:
                J = JS[c1]
                xc = xc_t[c1]
                d2 = dpl.tile([P, G, 2, J], BF16, tag="d2")
                nc.vector.tensor_sub(d2[:], xc[:, :, 2:4, 0:J],
                                     xc[:, :, 0:2, 1:J + 1])
                d3 = dpl.tile([P, G, 2, J], BF16, tag="d3")
                nc.vector.tensor_sub(d3[:], xc[:, :, 2:4, 1:J + 1],
                                     xc[:, :, 0:2, 1:J + 1])
                h = hp.tile([P, G, 4, J], BF16, tag="h")
                for m in range(G):
                    pA, pB, m1sb = [], [], []
                    # m1 = Wsum @ ph01 for both convs
                    for cv in range(2):
                        ps = psum.tile([P, 2, J], F32, space="PSUM", tag="ps")
                        for k in range(G):
                            nc.tensor.matmul(ps, wsum[cv][:, k, m, :],
                                             xc[:, k, 0:2, 1:J + 1],
                                             start=(k == 0), stop=(k == G - 1))
                        pA.append(ps)
                    # snapshot m1 (needed again for the ph23 outputs)
                    for cv in range(2):
                        t = m1p.tile([P, 2, J], BF16, tag="m1")
                        nc.vector.tensor_copy(t[:], pA[cv])
                        m1sb.append(t)
                    # m3 = Wtap1 @ d3 (fresh banks) -- gives the copies slack
                    for cv in range(2):
                        ps = psum.tile([P, 2, J], F32, space="PSUM", tag="ps")
                        for k in range(G):
                            nc.tensor.matmul(ps, wtap1[cv][:, k, m, :], d3[:, k],
                                             start=(k == 0), stop=(k == G - 1))
                        pB.append(ps)
                    # m2 = Wtap0 @ d2 accumulated in-place onto m1 -> y(ph01)
                    for cv in range(2):
                        for k in range(G):
                            nc.tensor.matmul(pA[cv], wtap0[cv][:, k, m, :],
                                             d2[:, k], start=False,
                                             stop=(k == G - 1),
                                             skip_group_check=True)
                    fg01, fg23 = [], []
                    for cv, afn, bn in ((0, AF.Tanh, "bf"),
                                        (1, AF.Sigmoid, "bg")):
                        t01 = fgp.tile([P, 2, J], BF16, tag="fg")
                        nc.scalar.activation(t01, pA[cv], afn,
                                             bias=bias_ap(bn, m))
                        p23 = fgp.tile([P, 2, J], BF16, tag="fg")
                        nc.vector.tensor_add(p23, pB[cv], m1sb[cv])
                        t23 = fgp.tile([P, 2, J], BF16, tag="fg")
                        nc.scalar.activation(t23, p23, afn,
                                             bias=bias_ap(bn, m))
                        fg01.append(t01)
                        fg23.append(t23)
                    nc.vector.tensor_mul(h[:, m, 0:2, :], fg01[0], fg01[1])
                    nc.vector.tensor_mul(h[:, m, 2:4, :], fg23[0], fg23[1])
                h_t[c1] = h

            # ---- stage 3: residual + skip projections (chunk c2) ----
            if 0 <= c2 < NCH:
                J, jo = JS[c2], JOS[c2]
                h = h_t.pop(c2)
                xc = xc_t.pop(c2)
                # per-half output DMAs on the last chunk start the final
                # stores before the second half's compute finishes
                fine = (c2 == NCH - 1)
                for m in range(G):
                    rt = rop.tile([P, 4, J], BF16, tag="r")
                    st = sop.tile([P, 4, J], BF16, tag="s")
                    off = 16 * jo + m * 4 * J
                    for h2 in range(2):
                        sl = slice(2 * h2, 2 * h2 + 2)
                        osl = slice(off + 2 * h2 * J, off + (2 * h2 + 2) * J)
                        pr = psum.tile([P, 2, J], F32, space="PSUM", tag="ps")
                        for k in range(G):
                            nc.tensor.matmul(pr, wr_sb[:, k, m, :],
                                             h[:, k, sl, :],
                                             start=(k == 0), stop=(k == G - 1))
                        nc.vector.scalar_tensor_tensor(
                            rt[:, sl, :], pr, bias_ap("br", m),
                            xc[:, m, sl, 1:J + 1], ALU.add, ALU.add)
                        pk = psum.tile([P, 2, J], F32, space="PSUM", tag="ps")
                        for k in range(G):
                            nc.tensor.matmul(pk, ws_sb[:, k, m, :],
                                             h[:, k, sl, :],
                                             start=(k == 0), stop=(k == G - 1))
                        nc.scalar.activation(st[:, sl, :], pk, AF.Identity,
                                             bias=bias_ap("bs", m))
                        if fine:
                            nc.scalar.dma_start(r_r[:, osl], rt[:, sl, :])
                            nc.sync.dma_start(s_r[:, osl], st[:, sl, :])
                    if not fine:
                        nc.scalar.dma_start(r_r[:, off:off + 4 * J], rt[:])
                        nc.sync.dma_start(s_r[:, off:off + 4 * J], st[:])

    nc.compile()
    return nc


def _get_nc():
    if "nc" not in _CACHE:
        _CACHE["nc"] = _build()
    return _CACHE["nc"]


def _wT1(w):
    # [Cout, Cin] -> lhsT layout [P(cin%P), G(cin//P), G(cout//P), P(cout%P)]
    return np.ascontiguousarray(
        np.asarray(w, dtype=np.float32).T.reshape(G, P, G, P)
        .transpose(1, 0, 2, 3).astype(ml_dtypes.bfloat16))


def _bias(b):
    return np.ascontiguousarray(np.asarray(b).reshape(G, P).T.astype(np.float32))


def _phase_perm(a):
    # [B, C, T] -> [B, P, 16Q] per-chunk-blocked: chunk c holds the
    # [G, 4, J_c] block (g, t%4, pair) flattened, chunks concatenated.
    ap = (np.asarray(a).reshape(B, G, P, Q, 4).transpose(0, 2, 1, 4, 3)
          .astype(ml_dtypes.bfloat16))          # [B, P, G, 4, Q]
    blocks = [
        np.ascontiguousarray(ap[:, :, :, :, JOS[c]:JOS[c] + JS[c]])
        .reshape(B, P, 16 * JS[c])
        for c in range(NCH)
    ]
    return np.ascontiguousarray(np.concatenate(blocks, axis=2))


def kernel(x, condition, wf, bf, wg, bg, wr, br, ws, bs, wc, bc):
    nc = _get_nc()
    x_p = _phase_perm(x)
    c_p = _phase_perm(condition)
    wf = np.asarray(wf, dtype=np.float32)
    wg = np.asarray(wg, dtype=np.float32)
    shared = {
        "wc": _wT1(np.asarray(wc)[:, :, 0]),
        "wfs": _wT1(wf[:, :, 0] + wf[:, :, 1]),
        "wf0": _wT1(wf[:, :, 0]),
        "wf1": _wT1(wf[:, :, 1]),
        "wgs": _wT1(wg[:, :, 0] + wg[:, :, 1]),
        "wg0": _wT1(wg[:, :, 0]),
        "wg1": _wT1(wg[:, :, 1]),
        "wr": _wT1(np.asarray(wr)[:, :, 0]),
        "ws": _wT1(np.asarray(ws)[:, :, 0]),
        "bias": np.ascontiguousarray(np.concatenate(
            [_bias(b) for b in (bc, bf, bg, br, bs)], axis=1)),
    }
    in_maps = [
        {"x": np.ascontiguousarray(x_p[i]), "cond": np.ascontiguousarray(c_p[i]),
         **shared}
        for i in range(B)
    ]
    res = run_bass_kernel_spmd(
        nc, in_maps, list(range(B)),
        trace=bool(os.environ.get("CC_KERNEL_TRACE")))
    _CACHE["last_results"] = res

    def unperm(name):
        # per-chunk-blocked [P, 16Q] -> [C, T]
        out = np.empty((B, P, G, 4, Q), dtype=np.float32)
        for i in range(B):
            flat = np.asarray(res.results[i][name]).astype(np.float32)
            for c in range(NCH):
                jo, J = JOS[c], JS[c]
                out[i, :, :, :, jo:jo + J] = (
                    flat[:, 16 * jo:16 * (jo + J)].reshape(P, G, 4, J))
        return np.ascontiguousarray(
            out.transpose(0, 2, 1, 4, 3).reshape(B, C, T))

    return unperm("res"), unperm("skip")

